# revision 6
# baseline (speedup 1.0000x reference)
"""Trainium2 Bass kernel for CustomBCELoss.

Reference semantics (per torch BCELoss with per-channel weighting):
    p, t flattened channel-first to (C=3, M=8388608)
    ones[c]   = count_nonzero(t[c])
    weight[c] = M / max(ones[c], 1)  if ones[c] > 0 else 1000.0
    bce[c]    = -mean(t*max(log p, -100) + (1-t)*max(log1p(-p), -100))
    out       = mean(weight * bce)

Since t ∈ {0,1}, the per-element term is log|p + t - 1|, and with
p ∈ [1e-4, 1-1e-4] the -100 clamp never fires: |p + t - 1| >= ~6e-5.

Single-stream encoding: p > 0 always, so its fp32 sign bit is free. The
host packs t there (p'' = +p if t==1 else -p, a lossless re-encoding of
the (p, t) pair), halving the HBM stream to 12.6 MB/core. On device:
  |p + t - 1| = p'' + (p'' < 0)   -- ONE fused DVE STT with src0 ==
      src1 == p_t: (p'' is_lt 0) add p''. The inner rounding is 2^-24
      absolute -> ~3e-7 relative error on the loss, far under
      tolerance.
  t = (p'' > 0) as bf16 {1.0, 0.0} -- plain DVE tensor_scalar (no
      accum_out: the accumulator read consumes a DVE port and forces
      1x; the plain op is eligible for 2x_2p perf mode).
  ones = bf16 matmul ones.T @ t into PSUM per segment (exact: products
      are 0/1, PSUM accumulates fp32).
8-way data-parallel over the flat element range; per-core pipeline over
[128, f] fp32 tiles. Engine split per tile: Sync issues all DMA; DVE
does is_gt (bf16 out) + the fused STT; PE counts; ACT does Ln with
fused per-partition accum_out. PE bf16 matmuls are full-clock; fp32r
matmuls and GpSimd DMA each statically derate all engine clocks 1.2x —
never use them. Tiles open at 1024 cols, cruise at 2048, taper
1024/1024/1024/512/512. A dummy Ln in the preamble pins the
natural_log table set. Results ship in readiness order (vsum bulk at
Ln12, counts after the last matmul+copies, vsum tail last).
Tiles never cross an (n, c) half-block boundary, so per-tile/per-segment
partials map 1:1 to channels on the host, which applies the tiny
weight/mean epilogue in float64.
"""

import numpy as np

import concourse.bacc as bacc
import concourse.bass as bass
import concourse.tile as tile
from concourse import mybir
from concourse.bass_utils import run_bass_kernel_spmd

N_CORES = 8
C = 3
SPATIAL = 128 * 128 * 128            # elements per (n, c) block
N_BATCH = 4
FULL = N_BATCH * C * SPATIAL         # 25_165_824 total elements
PER_CORE = FULL // N_CORES           # 3_145_728
P = 128
# Per-partition column counts per tile; sum must equal PER_CORE / P = 24576.
TILE_F = [1024, 2048, 2048, 2048, 1024,
          2048, 2048, 2048, 2048,
          2048, 2048, 1024, 1024, 1024, 512, 512]
NTILES = len(TILE_F)
TILE_ELEMS = [P * f for f in TILE_F]
assert sum(TILE_ELEMS) == PER_CORE
HALF_BLOCK_COLS = (SPATIAL // 2) // P          # 8192 cols per half-block
N_SEG = (PER_CORE // P) // HALF_BLOCK_COLS     # 3 segments per core
MM_N = 512                                      # matmul moving free dim
M_PER_CH = FULL // C                 # 8_388_608
EMPTY_WEIGHT = 1000.0
VS_SPLIT = 13                        # bulk/tail split for the vsum DMAs

_NC_CACHE = None


def _build_nc():
    nc = bacc.Bacc(
        "TRN2", target_bir_lowering=False, debug=False, num_devices=N_CORES
    )
    p_in = nc.declare_dram_parameter(
        "p_in", [PER_CORE], mybir.dt.float32, isOutput=False
    )
    vsum_out = nc.declare_dram_parameter(
        "vsum", [P, NTILES], mybir.dt.float32, isOutput=True
    )
    tsum_out = nc.declare_dram_parameter(
        "tsum", [1, N_SEG * MM_N], mybir.dt.float32, isOutput=True
    )

    seg_of_tile = []
    off = 0
    for f in TILE_F:
        assert off // HALF_BLOCK_COLS == (off + f - 1) // HALF_BLOCK_COLS
        seg_of_tile.append(off // HALF_BLOCK_COLS)
        off += f
    mm_total = {s: 0 for s in range(N_SEG)}
    for i, f in enumerate(TILE_F):
        mm_total[seg_of_tile[i]] += max(1, f // MM_N)

    with tile.TileContext(nc) as tc:
        with (
            tc.tile_pool(name="pp", bufs=10) as p_pool,
            tc.tile_pool(name="tp", bufs=4) as t_pool,
            tc.tile_pool(name="res", bufs=1) as res_pool,
            tc.tile_pool(name="ps", bufs=1, space="PSUM") as ps_pool,
        ):
            ones_t = res_pool.tile([P, 1], mybir.dt.bfloat16)
            nc.vector.memset(ones_t, 1.0)
            vsum_t = res_pool.tile([P, NTILES], mybir.dt.float32)
            cnt_sb = res_pool.tile([1, N_SEG * MM_N], mybir.dt.float32)
            # Dummy Ln pins the natural_log table set in the preamble.
            warm_t = res_pool.tile([P, 1], mybir.dt.float32)
            nc.vector.memset(warm_t, 1.0)
            nc.scalar.activation(
                out=warm_t, in_=warm_t, func=mybir.ActivationFunctionType.Ln
            )
            psum_seg = [
                ps_pool.tile(
                    [1, MM_N], mybir.dt.float32, tag=f"seg{s}", name=f"psum_seg{s}"
                )
                for s in range(N_SEG)
            ]
            mm_done = {s: 0 for s in range(N_SEG)}
            off = 0
            for i, f in enumerate(TILE_F):
                n = P * f
                p_src = p_in[off : off + n].rearrange("(p f) -> p f", p=P)
                off += n
                s = seg_of_tile[i]
                p_t = p_pool.tile([P, f], mybir.dt.float32, tag="p")
                nc.sync.dma_start(out=p_t, in_=p_src)
                # t = (p'' > 0) as bf16 {1.0, 0.0} for the count matmul.
                t_t = t_pool.tile([P, f], mybir.dt.bfloat16, tag="t")
                nc.vector.tensor_scalar(
                    out=t_t,
                    in0=p_t,
                    scalar1=0.0,
                    scalar2=None,
                    op0=mybir.AluOpType.is_gt,
                )
                w = min(MM_N, f)
                for j in range(max(1, f // MM_N)):
                    nc.tensor.matmul(
                        out=psum_seg[s][:, :w],
                        lhsT=ones_t[:, :],
                        rhs=t_t[:, j * w : (j + 1) * w],
                        start=(mm_done[s] == 0),
                        stop=(mm_done[s] == mm_total[s] - 1),
                    )
                    mm_done[s] += 1
                # |p + t - 1| = (p'' < 0) + p'', fused STT with src0 == src1,
                # in place into p_t.
                nc.vector.scalar_tensor_tensor(
                    out=p_t,
                    in0=p_t,
                    scalar=0.0,
                    in1=p_t,
                    op0=mybir.AluOpType.is_lt,
                    op1=mybir.AluOpType.add,
                )
                nc.scalar.activation(
                    out=p_t,
                    in_=p_t,
                    func=mybir.ActivationFunctionType.Ln,
                    accum_out=vsum_t[:, i : i + 1],
                )
            # DMA cannot read PSUM, so counts bounce through SBUF.
            for s in range(N_SEG):
                nc.vector.tensor_copy(
                    out=cnt_sb[:, s * MM_N : (s + 1) * MM_N],
                    in_=psum_seg[s],
                )
            # Ship results in readiness order so only a tiny vsum chunk
            # trails the last Ln.
            nc.sync.dma_start(
                out=vsum_out[:, :VS_SPLIT], in_=vsum_t[:, :VS_SPLIT]
            )
            nc.sync.dma_start(out=tsum_out[:], in_=cnt_sb)
            nc.sync.dma_start(
                out=vsum_out[:, VS_SPLIT:], in_=vsum_t[:, VS_SPLIT:]
            )
    nc.compile()
    return nc


def _get_nc():
    global _NC_CACHE
    if _NC_CACHE is None:
        _NC_CACHE = _build_nc()
    return _NC_CACHE


def _pack(input, target):
    """Lossless (p, t) -> p'' re-encoding: t into p's free sign bit."""
    p_flat = np.ascontiguousarray(input, dtype=np.float32).reshape(-1)
    t_flat = np.ascontiguousarray(target, dtype=np.float32).reshape(-1)
    p_bits = p_flat.view(np.uint32)
    sign = np.where(t_flat == 0.0, np.uint32(0x80000000), np.uint32(0))
    return (p_bits | sign).view(np.float32)


def _run_device(input, target, **spmd_kwargs):
    packed = _pack(input, target)
    in_maps = []
    for k in range(N_CORES):
        sl = slice(k * PER_CORE, (k + 1) * PER_CORE)
        in_maps.append({"p_in": packed[sl]})
    return run_bass_kernel_spmd(nc=_get_nc(), in_maps=in_maps,
                                core_ids=list(range(N_CORES)), **spmd_kwargs)


def _epilogue(results):
    sum_v = np.zeros(C, dtype=np.float64)
    ones = np.zeros(C, dtype=np.float64)
    for k in range(N_CORES):
        vs = results[k]["vsum"].astype(np.float64)   # [P, NTILES]
        ts = results[k]["tsum"].astype(np.float64)   # [1, N_SEG*MM_N]
        off = 0
        for i, n in enumerate(TILE_ELEMS):
            g = k * PER_CORE + off
            off += n
            ch = (g // SPATIAL) % C
            sum_v[ch] += vs[:, i].sum()
        for s in range(N_SEG):
            ch = ((k * N_SEG + s) // 2) % C
            ones[ch] += ts[0, s * MM_N : (s + 1) * MM_N].sum()
    total = float(M_PER_CH)
    weight = np.where(ones > 0, total / np.maximum(ones, 1.0), EMPTY_WEIGHT)
    bce = -sum_v / total
    return np.asarray((weight * bce).mean(), dtype=np.float32)


def kernel(input, target):
    res = _run_device(input, target)
    return _epilogue(res.results)


# revision 7
# speedup vs baseline: 1.0322x; 1.0322x over previous
"""Trainium2 Bass kernel for CustomBCELoss.

Reference semantics (per torch BCELoss with per-channel weighting):
    p, t flattened channel-first to (C=3, M=8388608)
    ones[c]   = count_nonzero(t[c])
    weight[c] = M / max(ones[c], 1)  if ones[c] > 0 else 1000.0
    bce[c]    = -mean(t*max(log p, -100) + (1-t)*max(log1p(-p), -100))
    out       = mean(weight * bce)

Since t ∈ {0,1}, the per-element term is log|p + t - 1|, and with
p ∈ [1e-4, 1-1e-4] the -100 clamp never fires: |p + t - 1| >= ~6e-5.

Single-stream encoding: p > 0 always, so its fp32 sign bit is free. The
host packs t there (p'' = +p if t==1 else -p, a lossless re-encoding of
the (p, t) pair), halving the HBM stream to 12.6 MB/core vs streaming
(p, t) separately. On device, per tile:
  u = |p + t - 1| = (p'' < 0) + p''   -- ONE fused DVE STT with
      src0 == src1 == p_t (the is_lt intermediate is the 1-t step).
      The inner rounding is 2^-24 absolute -> ~3e-7 relative error.
  Ln(u) with fused per-partition accum_out on ACT.
  count: the stream is DVE=1-pass/ACT=1-pass busy already, and a count
      is a third full pass; it is SPLIT across the two engines by tile
      to balance load (measured rates: DVE ~114 G elem/s, ACT ~131):
        * DVE tiles: tensor_scalar is_gt + accum_out (counts t==1).
        * ACT tiles: activation Sign + accum_out (sign ∈ {-1,+1} is in
          the natural_log table set, so no table switch); the host
          recovers ones = (accum + n)/2 exactly.
      Plain (no-accum) tensor_scalar can run a 2x DVE perf mode, but
      engaging it coincided with a measured 1.2x static clock derate
      on ALL engines (+ the accum read needs the port anyway) — the
      accum variants at 1x with full clock win.
PE/PSUM/GpSimd are never touched (fp32r matmuls and GpSimd DMA derate
all engine clocks 1.2x). Tiles open at 1024 cols, cruise at 2048,
taper 1024/1024/1024/512/512. A dummy Ln in the preamble pins the
natural_log table set. Results ship in readiness order.
Tiles never cross an (n, c) half-block boundary, so per-tile partials
map 1:1 to channels on the host, which applies the tiny weight/mean
epilogue in float64.
"""

import numpy as np

import concourse.bacc as bacc
import concourse.bass as bass
import concourse.tile as tile
from concourse import mybir
from concourse.bass_utils import run_bass_kernel_spmd

N_CORES = 8
C = 3
SPATIAL = 128 * 128 * 128            # elements per (n, c) block
N_BATCH = 4
FULL = N_BATCH * C * SPATIAL         # 25_165_824 total elements
PER_CORE = FULL // N_CORES           # 3_145_728
P = 128
# Per-partition column counts per tile; sum must equal PER_CORE / P = 24576.
TILE_F = [1024, 2048, 2048, 2048, 1024,
          2048, 2048, 2048, 2048,
          2048, 2048, 1024, 1024, 1024, 512, 512]
NTILES = len(TILE_F)
TILE_ELEMS = [P * f for f in TILE_F]
assert sum(TILE_ELEMS) == PER_CORE
HALF_BLOCK_COLS = (SPATIAL // 2) // P          # 8192 cols per half-block
M_PER_CH = FULL // C                 # 8_388_608
EMPTY_WEIGHT = 1000.0
VS_SPLIT = 13                        # bulk/tail split for the output DMAs
# Tiles whose count runs on DVE (is_gt+accum); the rest count on ACT
# (Sign+accum). Balance: DVE gets ~46% of count columns (11264/24576).
DVE_CNT_TILES = {0, 1, 2, 3, 4, 5, 6}

_NC_CACHE = None


def _build_nc():
    nc = bacc.Bacc(
        "TRN2", target_bir_lowering=False, debug=False, num_devices=N_CORES
    )
    p_in = nc.declare_dram_parameter(
        "p_in", [PER_CORE], mybir.dt.float32, isOutput=False
    )
    vsum_out = nc.declare_dram_parameter(
        "vsum", [P, NTILES], mybir.dt.float32, isOutput=True
    )
    cnt_out = nc.declare_dram_parameter(
        "cnt", [P, NTILES], mybir.dt.float32, isOutput=True
    )

    off = 0
    for f in TILE_F:
        assert off // HALF_BLOCK_COLS == (off + f - 1) // HALF_BLOCK_COLS
        off += f

    with tile.TileContext(nc) as tc:
        with (
            tc.tile_pool(name="pp", bufs=10) as p_pool,
            tc.tile_pool(name="up", bufs=4) as u_pool,
            tc.tile_pool(name="dp", bufs=2) as dump_pool,
            tc.tile_pool(name="res", bufs=1) as res_pool,
        ):
            vsum_t = res_pool.tile([P, NTILES], mybir.dt.float32)
            cnt_t = res_pool.tile([P, NTILES], mybir.dt.float32)
            # Dummy Ln pins the natural_log table set in the preamble
            # (it also contains Sign and Copy).
            warm_t = res_pool.tile([P, 1], mybir.dt.float32)
            nc.vector.memset(warm_t, 1.0)
            nc.scalar.activation(
                out=warm_t, in_=warm_t, func=mybir.ActivationFunctionType.Ln
            )
            off = 0
            for i, f in enumerate(TILE_F):
                n = P * f
                p_src = p_in[off : off + n].rearrange("(p f) -> p f", p=P)
                off += n
                p_t = p_pool.tile([P, f], mybir.dt.float32, tag="p")
                nc.sync.dma_start(out=p_t, in_=p_src)
                dump = dump_pool.tile([P, f], mybir.dt.bfloat16, tag="d")
                if i in DVE_CNT_TILES:
                    # count(t==1) per partition via the accumulator
                    # (op1=add is the reduce op).
                    nc.vector.tensor_scalar(
                        out=dump,
                        in0=p_t,
                        scalar1=0.0,
                        scalar2=None,
                        op0=mybir.AluOpType.is_gt,
                        op1=mybir.AluOpType.add,
                        accum_out=cnt_t[:, i : i + 1],
                    )
                else:
                    # accum = sum of sign(p'') = 2*ones - n; host recovers.
                    nc.scalar.activation(
                        out=dump,
                        in_=p_t,
                        func=mybir.ActivationFunctionType.Sign,
                        accum_out=cnt_t[:, i : i + 1],
                    )
                # u = |p + t - 1| = (p'' < 0) + p'', fused STT (src0==src1).
                u_t = u_pool.tile([P, f], mybir.dt.float32, tag="u")
                nc.vector.scalar_tensor_tensor(
                    out=u_t,
                    in0=p_t,
                    scalar=0.0,
                    in1=p_t,
                    op0=mybir.AluOpType.is_lt,
                    op1=mybir.AluOpType.add,
                )
                nc.scalar.activation(
                    out=u_t,
                    in_=u_t,
                    func=mybir.ActivationFunctionType.Ln,
                    accum_out=vsum_t[:, i : i + 1],
                )
            # Ship results in readiness order so only a tiny vsum chunk
            # trails the last Ln.
            nc.sync.dma_start(
                out=cnt_out[:, :VS_SPLIT], in_=cnt_t[:, :VS_SPLIT]
            )
            nc.sync.dma_start(
                out=vsum_out[:, :VS_SPLIT], in_=vsum_t[:, :VS_SPLIT]
            )
            nc.sync.dma_start(
                out=cnt_out[:, VS_SPLIT:], in_=cnt_t[:, VS_SPLIT:]
            )
            nc.sync.dma_start(
                out=vsum_out[:, VS_SPLIT:], in_=vsum_t[:, VS_SPLIT:]
            )
    nc.compile()
    return nc


def _get_nc():
    global _NC_CACHE
    if _NC_CACHE is None:
        _NC_CACHE = _build_nc()
    return _NC_CACHE


def _pack(input, target):
    """Lossless (p, t) -> p'' re-encoding: t into p's free sign bit."""
    p_flat = np.ascontiguousarray(input, dtype=np.float32).reshape(-1)
    t_flat = np.ascontiguousarray(target, dtype=np.float32).reshape(-1)
    p_bits = p_flat.view(np.uint32)
    sign = np.where(t_flat == 0.0, np.uint32(0x80000000), np.uint32(0))
    return (p_bits | sign).view(np.float32)


def _run_device(input, target, **spmd_kwargs):
    packed = _pack(input, target)
    in_maps = []
    for k in range(N_CORES):
        sl = slice(k * PER_CORE, (k + 1) * PER_CORE)
        in_maps.append({"p_in": packed[sl]})
    return run_bass_kernel_spmd(nc=_get_nc(), in_maps=in_maps,
                                core_ids=list(range(N_CORES)), **spmd_kwargs)


def _epilogue(results):
    sum_v = np.zeros(C, dtype=np.float64)
    ones = np.zeros(C, dtype=np.float64)
    for k in range(N_CORES):
        vs = results[k]["vsum"].astype(np.float64)   # [P, NTILES]
        ct = results[k]["cnt"].astype(np.float64)    # [P, NTILES]
        off = 0
        for i, n in enumerate(TILE_ELEMS):
            g = k * PER_CORE + off
            off += n
            ch = (g // SPATIAL) % C
            sum_v[ch] += vs[:, i].sum()
            if i in DVE_CNT_TILES:
                ones[ch] += ct[:, i].sum()
            else:
                # accum was sum of sign = 2*ones_tile - n_tile
                ones[ch] += (ct[:, i].sum() + n) / 2.0
    total = float(M_PER_CH)
    weight = np.where(ones > 0, total / np.maximum(ones, 1.0), EMPTY_WEIGHT)
    bce = -sum_v / total
    return np.asarray((weight * bce).mean(), dtype=np.float32)


def kernel(input, target):
    res = _run_device(input, target)
    return _epilogue(res.results)


# revision 9
# speedup vs baseline: 1.1405x; 1.1049x over previous
"""Trainium2 Bass kernel for CustomBCELoss.

Reference semantics (per torch BCELoss with per-channel weighting):
    p, t flattened channel-first to (C=3, M=8388608)
    ones[c]   = count_nonzero(t[c])
    weight[c] = M / max(ones[c], 1)  if ones[c] > 0 else 1000.0
    bce[c]    = -mean(t*max(log p, -100) + (1-t)*max(log1p(-p), -100))
    out       = mean(weight * bce)

Since t ∈ {0,1}, the per-element term is log|p + t - 1|, and with
p ∈ [1e-4, 1-1e-4] the -100 clamp never fires: |p + t - 1| >= ~6e-5.

Single-stream encoding: p > 0 always, so its fp32 sign bit is free. The
host packs t there (p'' = +p if t==1 else -p, a lossless re-encoding of
the (p, t) pair), halving the HBM stream to 12.6 MB/core vs streaming
(p, t) separately. On device, per tile:
  u = |p + t - 1| = (p'' < 0) + p''   -- ONE fused DVE STT with
      src0 == src1 == p_t (the is_lt intermediate is the 1-t step).
      The inner rounding is 2^-24 absolute -> ~3e-7 relative error.
  Ln(u) with fused per-partition accum_out on ACT.
  count: the stream is DVE=1-pass/ACT=1-pass busy already, and a count
      is a third full pass; it is SPLIT across the two engines by tile
      to balance load (measured rates: DVE ~114 G elem/s, ACT ~131):
        * DVE tiles: tensor_scalar is_gt + accum_out (counts t==1).
        * ACT tiles: activation Sign + accum_out (sign ∈ {-1,+1} is in
          the natural_log table set, so no table switch); the host
          recovers ones = (accum + n)/2 exactly.
      Plain (no-accum) tensor_scalar can run a 2x DVE perf mode, but
      engaging it coincided with a measured 1.2x static clock derate
      on ALL engines (+ the accum read needs the port anyway) — the
      accum variants at 1x with full clock win.
PE/PSUM/GpSimd are never touched (fp32r matmuls and GpSimd DMA derate
all engine clocks 1.2x). Tiles open at 1024 cols, cruise at 2048,
taper 1024/1024/1024/512/512. A dummy Ln in the preamble pins the
natural_log table set. Results ship in readiness order.
Tiles never cross an (n, c) half-block boundary, so per-tile partials
map 1:1 to channels on the host, which applies the tiny weight/mean
epilogue in float64.
"""

import numpy as np

import concourse.bacc as bacc
import concourse.bass as bass
import concourse.tile as tile
from concourse import mybir
from concourse.bass_utils import run_bass_kernel_spmd

N_CORES = 8
C = 3
SPATIAL = 128 * 128 * 128            # elements per (n, c) block
N_BATCH = 4
FULL = N_BATCH * C * SPATIAL         # 25_165_824 total elements
PER_CORE = FULL // N_CORES           # 3_145_728
P = 128
# Per-partition column counts per tile; sum must equal PER_CORE / P = 24576.
TILE_F = [1024, 2048, 2048, 2048, 1024,
          2048, 2048, 2048, 2048,
          2048, 2048, 1024, 1024, 1024, 512, 512]
NTILES = len(TILE_F)
TILE_ELEMS = [P * f for f in TILE_F]
assert sum(TILE_ELEMS) == PER_CORE
HALF_BLOCK_COLS = (SPATIAL // 2) // P          # 8192 cols per half-block
M_PER_CH = FULL // C                 # 8_388_608
EMPTY_WEIGHT = 1000.0
VS_SPLIT = 13                        # bulk/tail split for the output DMAs
# Tiles whose count runs on DVE (is_gt+accum); the rest count on ACT
# (Sign+accum). Balance: DVE gets ~48% of count columns (11776/24576),
# interleaved in small irregular clusters (runs of 1-3) so neither
# engine accumulates a long single-engine-bound phase (a fully
# clustered split measured ~18 us of pipeline bubbles), while avoiding
# the strict-alternation pattern that trips the 1.2x clock derate.
DVE_CNT_TILES = {0, 1, 4, 5, 6, 10, 11, 14}

_NC_CACHE = None


def _build_nc():
    nc = bacc.Bacc(
        "TRN2", target_bir_lowering=False, debug=False, num_devices=N_CORES
    )
    p_in = nc.declare_dram_parameter(
        "p_in", [PER_CORE], mybir.dt.float32, isOutput=False
    )
    vsum_out = nc.declare_dram_parameter(
        "vsum", [P, NTILES], mybir.dt.float32, isOutput=True
    )
    cnt_out = nc.declare_dram_parameter(
        "cnt", [P, NTILES], mybir.dt.float32, isOutput=True
    )

    off = 0
    for f in TILE_F:
        assert off // HALF_BLOCK_COLS == (off + f - 1) // HALF_BLOCK_COLS
        off += f

    with tile.TileContext(nc) as tc:
        with (
            tc.tile_pool(name="pp", bufs=12) as p_pool,
            tc.tile_pool(name="up", bufs=6) as u_pool,
            tc.tile_pool(name="dp", bufs=2) as dump_pool,
            tc.tile_pool(name="res", bufs=1) as res_pool,
        ):
            vsum_t = res_pool.tile([P, NTILES], mybir.dt.float32)
            cnt_t = res_pool.tile([P, NTILES], mybir.dt.float32)
            # Dummy Ln pins the natural_log table set in the preamble
            # (it also contains Sign and Copy).
            warm_t = res_pool.tile([P, 1], mybir.dt.float32)
            nc.vector.memset(warm_t, 1.0)
            nc.scalar.activation(
                out=warm_t, in_=warm_t, func=mybir.ActivationFunctionType.Ln
            )
            off = 0
            for i, f in enumerate(TILE_F):
                n = P * f
                p_src = p_in[off : off + n].rearrange("(p f) -> p f", p=P)
                off += n
                p_t = p_pool.tile([P, f], mybir.dt.float32, tag="p")
                nc.sync.dma_start(out=p_t, in_=p_src)
                dump = dump_pool.tile([P, f], mybir.dt.bfloat16, tag="d")
                if i in DVE_CNT_TILES:
                    # count(t==1) per partition via the accumulator
                    # (op1=add is the reduce op).
                    nc.vector.tensor_scalar(
                        out=dump,
                        in0=p_t,
                        scalar1=0.0,
                        scalar2=None,
                        op0=mybir.AluOpType.is_gt,
                        op1=mybir.AluOpType.add,
                        accum_out=cnt_t[:, i : i + 1],
                    )
                else:
                    # accum = sum of sign(p'') = 2*ones - n; host recovers.
                    nc.scalar.activation(
                        out=dump,
                        in_=p_t,
                        func=mybir.ActivationFunctionType.Sign,
                        accum_out=cnt_t[:, i : i + 1],
                    )
                # u = |p + t - 1| = (p'' < 0) + p'', fused STT (src0==src1).
                u_t = u_pool.tile([P, f], mybir.dt.float32, tag="u")
                nc.vector.scalar_tensor_tensor(
                    out=u_t,
                    in0=p_t,
                    scalar=0.0,
                    in1=p_t,
                    op0=mybir.AluOpType.is_lt,
                    op1=mybir.AluOpType.add,
                )
                nc.scalar.activation(
                    out=u_t,
                    in_=u_t,
                    func=mybir.ActivationFunctionType.Ln,
                    accum_out=vsum_t[:, i : i + 1],
                )
            # Ship results in readiness order so only a tiny vsum chunk
            # trails the last Ln.
            nc.sync.dma_start(
                out=cnt_out[:, :VS_SPLIT], in_=cnt_t[:, :VS_SPLIT]
            )
            nc.sync.dma_start(
                out=vsum_out[:, :VS_SPLIT], in_=vsum_t[:, :VS_SPLIT]
            )
            nc.sync.dma_start(
                out=cnt_out[:, VS_SPLIT:], in_=cnt_t[:, VS_SPLIT:]
            )
            nc.sync.dma_start(
                out=vsum_out[:, VS_SPLIT:], in_=vsum_t[:, VS_SPLIT:]
            )
    nc.compile()
    return nc


def _get_nc():
    global _NC_CACHE
    if _NC_CACHE is None:
        _NC_CACHE = _build_nc()
    return _NC_CACHE


def _pack(input, target):
    """Lossless (p, t) -> p'' re-encoding: t into p's free sign bit."""
    p_flat = np.ascontiguousarray(input, dtype=np.float32).reshape(-1)
    t_flat = np.ascontiguousarray(target, dtype=np.float32).reshape(-1)
    p_bits = p_flat.view(np.uint32)
    sign = np.where(t_flat == 0.0, np.uint32(0x80000000), np.uint32(0))
    return (p_bits | sign).view(np.float32)


def _run_device(input, target, **spmd_kwargs):
    packed = _pack(input, target)
    in_maps = []
    for k in range(N_CORES):
        sl = slice(k * PER_CORE, (k + 1) * PER_CORE)
        in_maps.append({"p_in": packed[sl]})
    return run_bass_kernel_spmd(nc=_get_nc(), in_maps=in_maps,
                                core_ids=list(range(N_CORES)), **spmd_kwargs)


def _epilogue(results):
    sum_v = np.zeros(C, dtype=np.float64)
    ones = np.zeros(C, dtype=np.float64)
    for k in range(N_CORES):
        vs = results[k]["vsum"].astype(np.float64)   # [P, NTILES]
        ct = results[k]["cnt"].astype(np.float64)    # [P, NTILES]
        off = 0
        for i, n in enumerate(TILE_ELEMS):
            g = k * PER_CORE + off
            off += n
            ch = (g // SPATIAL) % C
            sum_v[ch] += vs[:, i].sum()
            if i in DVE_CNT_TILES:
                ones[ch] += ct[:, i].sum()
            else:
                # accum was sum of sign = 2*ones_tile - n_tile
                ones[ch] += (ct[:, i].sum() + n) / 2.0
    total = float(M_PER_CH)
    weight = np.where(ones > 0, total / np.maximum(ones, 1.0), EMPTY_WEIGHT)
    bce = -sum_v / total
    return np.asarray((weight * bce).mean(), dtype=np.float32)


def kernel(input, target):
    res = _run_device(input, target)
    return _epilogue(res.results)
